# revision 45
# baseline (speedup 1.0000x reference)
"""Trainium2 Bass kernel for nn_ContrastiveLearning (self-contained).

kernel(**inputs) takes the FULL unsharded inputs (as produced by the
problem's setup_inputs) and returns (logits_per_img, logits_per_depth),
each [4, 100, 100] fp32.

Sharding (communication-free): 8 NeuronCores, core c = (batch b=c//2,
half=c%2). Each core computes logits rows [50h:50h+50] of batch b's
100x100 contrastive matrix, which needs e1 for its 50 own patches
(img branch of feat_c1[b], 5 patch-row slabs) plus e2 for ALL 100
patches (depth branch of feat_c2[b], 10 slabs). Features ship as bf16
(half the DMA bytes; rel err ~4e-3, well under the 2e-2 gate).

The conv1x1 runs transposed-stationary on the PE: the slab pixels are
the stationary lhsT ([128c, 128pix] slice) and the conv weight column
is the moving rhs ([128c, 1]), accumulating over the two channel
halves, so each matmul lands a full xT column [128,1] directly in
PSUM. This removes the [1, N]-layout conv output entirely: no
1-partition evacuation ops, no compaction DMA, no PE transpose.
ReLU+bias happen on the 128-partition xT tile during PSUM->SBUF evac.

Patches are processed in groups in slab-arrival order (own 20+30 |
full 3x early slabs | f3-f5 | f6,f8,f9,f7 last with f7 split across
all 3 DMA queues); the full-branch eT columns are permuted accordingly
and the host unpermutes the logits columns. LayerNorm's rstd is
computed entirely on DVE (reciprocal + min-of-two-chords linear seed +
2 Newton iterations) so no LN step ever queues behind the scalar
engine's DMA backlog; the logits ship in two column strips. No
collectives / cross-core traffic at all (an AllGather costs a flat
15us in this cost regime). sqrt(exp(logit_scale)) is folded into both
branches' LayerNorm affine on the host.
"""
import os
import numpy as np
import ml_dtypes
import concourse.bass as bass
import concourse.bacc as bacc
import concourse.mybir as mybir
import concourse.tile as tile
from concourse.bass_utils import run_bass_kernel_spmd


F32 = mybir.dt.float32
BF16 = mybir.dt.bfloat16
AF = mybir.ActivationFunctionType
ALU = mybir.AluOpType

NV = NH = 10          # patch grid
NP = NV * NH          # 100 patches per batch-modality
CPS = 16
ENC = 128
LN_EPS = 1e-5
HALF = NP // 2        # 50 logits rows per core
NTOT = HALF + NP      # 150 patches per core (50 own + 100 full)

# wpack (bf16): 0:512 own w1t | 512:768 own w2t | 768:1280 full w1t |
#   1280:1536 full w2t.  wsmall (f32): own conv_b, g*s, b*s, then full.

# per-slab chunks (LayerNorm is per-patch, so each 10-patch slab is an
# independent pipeline unit). eT cols: own o_i -> 10i; full f0-f6 ->
# 50+10i, f8 -> 120, f9 -> 130, f7 -> 140 (f7 lands last; host
# unpermutes the logits columns via FULL_PATCH_ORDER).
SLAB_COL = {**{f"o{i}": 10 * i for i in range(5)},
            **{f"f{i}": 50 + 10 * i for i in range(7)},
            "f8": 120, "f9": 130, "f7": 140}
SLAB_N = 10
FULL_PATCH_ORDER = (list(range(0, 70)) + list(range(80, 100))
                    + list(range(70, 80)))
# DVE has no pow/rsqrt in the real ISA. Default: DVE-only reciprocal +
# linear seed + 3 Newton iterations (converges for v in [0.005, 0.5];
# the observed pre-LN variance range is [0.024, 0.139]). BASS_RSTD=sqrt
# selects the scalar-engine Sqrt path instead.
RSTD_SQRT = os.environ.get("BASS_RSTD") == "sqrt"
# rsqrt seed: min of two chords of sqrt(u), u = 1/v, fit on u in [5,15]
# and [15,55]; min-of-chords underestimates (concave), so 2 Newton
# iterations converge to <1e-3 over the whole observed variance range.
RSQRT_CHORDS = [(1.359, 0.1754), (2.093, 0.1142), (3.144, 0.0767)]


def build_kernel(nc, n_cores=8):
    # feats are host-permuted to patch-major, pixel-contiguous layout
    # [u, p, patch, 256] so the conv's stationary AP is a single free dim
    fown = nc.dram_tensor("fown", [2, 128, HALF, 2 * ENC], BF16,
                          kind="ExternalInput")
    ffull = nc.dram_tensor("ffull", [2, 128, NP, 2 * ENC], BF16,
                           kind="ExternalInput")
    convw = nc.dram_tensor("convw", [128, 4], BF16, kind="ExternalInput")
    wpack = nc.dram_tensor("wpack", [128, 1536], BF16, kind="ExternalInput")
    wsmall = nc.dram_tensor("wsmall", [128, 6], F32, kind="ExternalInput")
    logits = nc.dram_tensor("logits", [HALF, NP], F32, kind="ExternalOutput")

    with tile.TileContext(nc) as tc:
        with (
            tc.tile_pool(name="slab", bufs=15) as slab_pool,
            tc.tile_pool(name="cst", bufs=1) as cst,
            tc.tile_pool(name="work", bufs=1) as work,
            tc.tile_pool(name="rot", bufs=7) as rot,
            tc.tile_pool(name="cv", bufs=3, space="PSUM") as ps_cv,
            tc.tile_pool(name="mm", bufs=4, space="PSUM") as ps_mm,
            tc.tile_pool(name="lg", bufs=1, space="PSUM") as ps_lg,
        ):
            convw_s = cst.tile([128, 4], BF16, tag="convw")
            wp_s = cst.tile([128, 1536], BF16, tag="wpack")
            ws_s = cst.tile([128, 6], F32, tag="wsmall")
            ones_col = cst.tile([128, 1], F32, tag="onec")
            ones_row = cst.tile([1, 128], F32, tag="oner")
            nones_row = cst.tile([1, 128], F32, tag="noner")
            scr = cst.tile([1, 8], F32, tag="scr")
            nc.gpsimd.memset(ones_col[:], 1.0 / ENC)  # stats -> means
            nc.gpsimd.memset(ones_row[:], 1.0)
            nc.gpsimd.memset(nones_row[:], -1.0)
            nc.scalar.dma_start(convw_s[:], convw[:])
            if RSTD_SQRT:  # preload the Sqrt activation table early
                nc.scalar.activation(scr[0:1, 0:1], ones_row[0:1, 0:1],
                                     AF.Sqrt)
            nc.scalar.dma_start(wp_s[:], wpack[:])
            nc.scalar.dma_start(ws_s[:], wsmall[:])
            w1t = [wp_s[:, 0:512], wp_s[:, 768:1280]]        # per branch
            w2t = [wp_s[:, 512:768], wp_s[:, 1280:1536]]
            cb_s = [ws_s[:, 0:1], ws_s[:, 3:4]]              # conv bias
            g_s = [ws_s[:, 1:2], ws_s[:, 4:5]]
            b_s = [ws_s[:, 2:3], ws_s[:, 5:6]]

            # persistent sbuf tiles
            xT = [work.tile([128, NTOT], BF16, tag=f"xT{v}", name=f"xT{v}")
                  for v in range(2)]
            hT = [work.tile([128, NTOT], BF16, tag=f"hT{t}", name=f"hT{t}")
                  for t in range(2)]
            yT = work.tile([128, NTOT], F32, tag="yT")
            sqT = work.tile([128, NTOT], F32, tag="sqT")
            eT = work.tile([128, NTOT], F32, tag="eT")
            ego = work.tile([128, HALF], F32, tag="ego")   # g_full * eT_own
            wrow = work.tile([1, 128], F32, tag="wrow")    # [-g^T eTo | b^T eTo]

            slabs = {}
            for nm in [f"o{i}" for i in range(5)] + [f"f{i}" for i in range(10)]:
                slabs[nm] = slab_pool.tile([128, 2, SLAB_N, 2 * ENC], BF16,
                                           tag="slab", name=f"st_{nm}")

            def slab_dma(eng, nm, n0=0, n1=SLAB_N):
                src = fown if nm[0] == "o" else ffull
                s = int(nm[1:])
                eng.dma_start(
                    slabs[nm][:, :, n0:n1, :],
                    src[:, :, s * SLAB_N + n0:s * SLAB_N + n1, :].rearrange(
                        "u p n x -> p u n x"))

            # queue schedule (slab 3948ns, f7 split across all queues):
            #  SP:   o0 o2 f1 f4 f8 f7[0:4]             ~20.9us busy
            #  Pool: o1 o3 f2 f5 f9 f7[4:8]              ~20.9us
            #  Act:  cw wp wsm o4 f0 f3 f6 f7[8:16]      ~21.2us
            for nm in ["o0", "o2", "f1", "f4", "f8"]:
                slab_dma(nc.sync, nm)
            slab_dma(nc.sync, "f7", 0, 3)
            for nm in ["o1", "o3", "f2", "f5", "f9"]:
                slab_dma(nc.gpsimd, nm)
            slab_dma(nc.gpsimd, "f7", 3, 6)
            for nm in ["o4", "f0", "f3", "f6"]:
                slab_dma(nc.scalar, nm)
            slab_dma(nc.scalar, "f7", 6, 10)

            # processing groups: slabs with contiguous eT columns, convolved
            # into one shared psum pair per group, then batched
            # evac/MLP/stats/LN over the whole column range.
            def run_group(subs, ve, cvp=None, raw=False, prows=False):
                cvp = cvp or ps_cv
                nms = [nm for sub in subs for nm in sub]
                n = SLAB_N * len(nms)
                c0 = min(SLAB_COL[nm] for nm in nms)
                br = 0 if nms[0][0] == "o" else 1
                J = slice(c0, c0 + n)
                rb = ps_mm.tile([128, 512], F32, tag="mm", name=f"rb_{nms[0]}")
                off = 0
                for sub in subs:
                    ns = SLAB_N * len(sub)
                    cs = c0 + off
                    Js = slice(cs, cs + ns)
                    pxg = [cvp.tile([128, 512], F32, tag="cv",
                                    name=f"px_{sub[0]}{v}") for v in range(2)]
                    for i, nm in enumerate(sub):
                        st = slabs[nm]
                        for j in range(SLAB_N):
                            for v in range(2):
                                for u in range(2):
                                    # one accumulation group per column: safe
                                    # under any scheduler order (start only
                                    # lazily zeroes; reads see raw psum)
                                    nc.tensor.matmul(
                                        pxg[v][:, 10 * i + j:10 * i + j + 1],
                                        st[:, u, j, 128 * v:128 * (v + 1)],
                                        convw_s[:, 2 * br + u:2 * br + u + 1],
                                        start=(u == 0), stop=(u == 1),
                                    )
                    for v in range(2):
                        ve.tensor_scalar(xT[v][:, Js], pxg[v][:, 0:ns],
                                         cb_s[br][:], 0.0, ALU.add, ALU.max)
                    for t in range(2):
                        ph = ps_mm.tile([128, 512], F32, tag="mm",
                                        name=f"ph{t}_{sub[0]}")
                        for v in range(2):
                            nc.tensor.matmul(
                                ph[:, 0:ns],
                                w1t[br][:, 256 * v + 128 * t:
                                          256 * v + 128 * t + 128],
                                xT[v][:, Js], start=(v == 0), stop=(v == 1))
                        ve.tensor_scalar_max(hT[t][:, Js], ph[:, 0:ns], 0.0)
                    py = ps_mm.tile([128, 512], F32, tag="mm",
                                    name=f"py_{sub[0]}")
                    for t in range(2):
                        nc.tensor.matmul(py[:, 0:ns],
                                         w2t[br][:, 128 * t:128 * t + 128],
                                         hT[t][:, Js], start=(t == 0),
                                         stop=(t == 1))
                    ve.tensor_copy(yT[:, Js], py[:, 0:ns])
                    (nc.gpsimd if raw else ve).tensor_tensor(
                        sqT[:, Js], yT[:, Js], yT[:, Js], ALU.mult)
                    nc.tensor.matmul(rb[0:1, off:off + ns], ones_col[:],
                                     yT[:, Js], start=True, stop=True)
                    nc.tensor.matmul(rb[0:1, n + off:n + off + ns],
                                     ones_col[:], sqT[:, Js],
                                     start=True, stop=True)
                    off += ns
                rw = rot.tile([1, 320], F32, tag="rw", name=f"rw_{nms[0]}")
                qrow = rb[0:1, n:2 * n]
                mrow = rw[0:1, 0:n]
                rstd = rw[0:1, 120:120 + n]
                t1, veps = rw[0:1, 184:184 + n], rw[0:1, 248:248 + n]
                ve.tensor_copy(mrow, rb[0:1, 0:n])   # mean to SBUF
                ve.tensor_tensor(t1, mrow, mrow, ALU.mult)
                # eps (1e-5) is <0.05% of the observed minimum variance
                # (0.024); folding it away costs ~2e-4 rel on rstd
                ve.tensor_tensor(veps, qrow, t1, ALU.subtract)
                nc.vector.reciprocal(t1, veps)
                if raw or prows:
                    # late group: single-chord seed + 2 Newton iterations,
                    # all on the idle gpsimd queue (TensorTensor min is not
                    # legal on Pool, so no min-of-chords here)
                    re, iters = nc.gpsimd, 2
                    re.tensor_scalar(rstd, t1, 0.1036, 1.718,
                                     ALU.mult, ALU.add)
                else:
                    re, iters = ve, 1
                    s2 = rw[0:1, 60:60 + n]
                    a0, b0 = RSQRT_CHORDS[0]
                    ve.tensor_scalar(rstd, t1, b0, a0, ALU.mult, ALU.add)
                    for a, b in RSQRT_CHORDS[1:]:
                        ve.tensor_scalar(s2, t1, b, a, ALU.mult, ALU.add)
                        ve.tensor_tensor(rstd, rstd, s2, ALU.min)
                for _ in range(iters):
                    re.tensor_tensor(t1, rstd, rstd, ALU.mult)
                    re.tensor_tensor(t1, t1, veps, ALU.mult)
                    re.tensor_scalar(t1, t1, -0.5, 1.5, ALU.mult, ALU.add)
                    re.tensor_tensor(rstd, rstd, t1, ALU.mult)
                re.tensor_tensor(mrow, mrow, rstd, ALU.mult)  # mean*rstd
                nc.tensor.matmul(rb[:, 128:128 + n], ones_row[:], rstd,
                                 start=True, stop=True)
                if raw:
                    # eT holds y*rstd only; -mean*rstd and +b are folded
                    # into the logits matmul as rank-1 accumulations
                    ve.tensor_tensor(eT[:, J], yT[:, J],
                                     rb[:, 128:128 + n], ALU.mult)
                    return rw
                nc.tensor.matmul(rb[:, 192:192 + n], nones_row[:],
                                 mrow, start=True, stop=True)
                ve.tensor_tensor(yT[:, J], yT[:, J],
                                 rb[:, 128:128 + n], ALU.mult)
                ve.tensor_tensor(yT[:, J], yT[:, J],
                                 rb[:, 192:192 + n], ALU.add)
                ve.tensor_scalar(eT[:, J], yT[:, J], g_s[br][:],
                                 b_s[br][:], ALU.mult, ALU.add)
                return rw

            # early groups ride DVE; late groups ride gpsimd, whose DMA
            # queue drains just before their slabs land
            run_group([["o0", "o1"]], nc.vector)
            run_group([["o2", "o3", "o4"]], nc.vector)
            # folded-logits precomputes (own eT ready; full-branch g/b)
            nc.vector.tensor_scalar(ego[:, 0:HALF], eT[:, 0:HALF],
                                    g_s[1][:], None, ALU.mult)
            pwr = ps_mm.tile([128, 512], F32, tag="mm", name="pwr")
            nc.tensor.matmul(pwr[0:1, 0:HALF], g_s[1], eT[:, 0:HALF],
                             start=True, stop=True)
            nc.tensor.matmul(pwr[0:1, 64:64 + HALF], b_s[1], eT[:, 0:HALF],
                             start=True, stop=True)
            nc.vector.tensor_scalar(wrow[0:1, 0:HALF], pwr[0:1, 0:HALF],
                                    -1.0, None, ALU.mult)
            nc.vector.tensor_copy(wrow[0:1, 64:64 + HALF],
                                  pwr[0:1, 64:64 + HALF])
            run_group([["f0", "f1", "f2"]], nc.vector)
            run_group([["f3", "f4", "f5"]], nc.vector, prows=True)

            rwL = run_group([["f6", "f8", "f9", "f7"]], nc.vector, raw=True)

            # logits part 1: device cols 0:60 (f0-f5)
            pL = ps_lg.tile([HALF, NP], F32, tag="lg")
            L_s = work.tile([HALF, NP], F32, tag="Ls")
            nc.tensor.matmul(pL[:, 0:60], eT[:, 0:HALF], eT[:, HALF:HALF + 60],
                             start=True, stop=True)
            nc.scalar.copy(L_s[:, 0:60], pL[:, 0:60])
            nc.scalar.dma_start(logits[:, 0:60], L_s[:, 0:60])

            nc.tensor.matmul(pL[:, 60:NP], wrow[0:1, 64:64 + HALF],
                             ones_row[0:1, 0:40], start=True, stop=False)
            nc.tensor.matmul(pL[:, 60:NP], wrow[0:1, 0:HALF],
                             rwL[0:1, 0:40], start=False, stop=False)
            nc.tensor.matmul(pL[:, 60:NP], ego[:, 0:HALF],
                             eT[:, HALF + 60:NTOT], start=False, stop=True)
            nc.vector.tensor_copy(L_s[:, 60:NP], pL[:, 60:NP])
            nc.sync.dma_start(logits[:, 60:NP], L_s[:, 60:NP])

    nc.compile()
    return nc


def host_inputs_for_core(core, inputs):
    """Build the per-core in_map from the full problem inputs dict."""
    b, half = core // 2, core % 2
    f1 = np.asarray(inputs["feat_c1"])[b]          # [256,160,160] img branch
    f2 = np.asarray(inputs["feat_c2"])[b]          # depth branch
    fown = f1[:, half * 80:half * 80 + 80, :]      # own 5 patch-rows
    s_half = np.sqrt(np.exp(np.float32(np.asarray(inputs["logit_scale"]))))

    def conv2(pre):
        return np.asarray(inputs[pre + "conv_w"]).reshape(2, 128).T  # [128,2]

    convw = np.concatenate([conv2("img_"), conv2("depth_")], axis=1)

    def w1t(pre):
        w1 = np.asarray(inputs[pre + "w1"])        # [256,256]
        return np.ascontiguousarray(
            w1.reshape(2, 128, 2, 128).transpose(3, 2, 0, 1).reshape(128, 512))

    def w2t(pre):
        w2 = np.asarray(inputs[pre + "w2"])        # [128,256]
        return np.ascontiguousarray(
            w2.reshape(128, 2, 128).transpose(2, 1, 0).reshape(128, 256))

    def col(val):
        return np.full((128, 1), np.float32(val), np.float32)

    def gcol(pre, nm):
        return (np.asarray(inputs[pre + nm]) * s_half).reshape(128, 1)

    wpack = np.concatenate([
        w1t("img_"), w2t("img_"), w1t("depth_"), w2t("depth_"),
    ], axis=1)
    wsmall = np.concatenate([
        col(np.asarray(inputs["img_conv_b"])[0]),
        gcol("img_", "ln_g").astype(np.float32),
        gcol("img_", "ln_b").astype(np.float32),
        col(np.asarray(inputs["depth_conv_b"])[0]),
        gcol("depth_", "ln_g").astype(np.float32),
        gcol("depth_", "ln_b").astype(np.float32),
    ], axis=1).astype(np.float32)
    def patchmaj(f, nr):
        # [256, nr*16, 160] -> [2, 128, nr*10 patches, 256 pixels]
        t = f.reshape(2, 128, nr, CPS, NH, CPS).transpose(0, 1, 2, 4, 3, 5)
        return np.ascontiguousarray(t.reshape(2, 128, nr * NH, 2 * ENC))

    return {
        "fown": patchmaj(fown, 5).astype(ml_dtypes.bfloat16),
        "ffull": patchmaj(f2, 10).astype(ml_dtypes.bfloat16),
        "convw": convw.astype(ml_dtypes.bfloat16),
        "wpack": np.ascontiguousarray(wpack).astype(ml_dtypes.bfloat16),
        "wsmall": np.ascontiguousarray(wsmall),
    }


_NC_CACHE = {}


def _get_nc():
    if "nc" not in _NC_CACHE:
        nc = bacc.Bacc("TRN2", target_bir_lowering=False, num_devices=8)
        build_kernel(nc)
        _NC_CACHE["nc"] = nc
    return _NC_CACHE["nc"]


def kernel(**inputs):
    nc = _get_nc()
    in_maps = [host_inputs_for_core(c, inputs) for c in range(8)]
    res = run_bass_kernel_spmd(nc, in_maps, list(range(8)))
    perm = np.asarray(FULL_PATCH_ORDER)
    logits_img = np.empty((4, NP, NP), np.float32)
    for b in range(4):
        dev = np.concatenate([np.asarray(res.results[2 * b]["logits"]),
                              np.asarray(res.results[2 * b + 1]["logits"])],
                             axis=0).astype(np.float32)
        logits_img[b][:, perm] = dev
    logits_depth = np.ascontiguousarray(logits_img.transpose(0, 2, 1))
    return logits_img, logits_depth


# revision 46
# speedup vs baseline: 1.0089x; 1.0089x over previous
"""Trainium2 Bass kernel for nn_ContrastiveLearning (self-contained).

kernel(**inputs) takes the FULL unsharded inputs (as produced by the
problem's setup_inputs) and returns (logits_per_img, logits_per_depth),
each [4, 100, 100] fp32.

Sharding (communication-free): 8 NeuronCores, core c = (batch b=c//2,
half=c%2). Each core computes logits rows [50h:50h+50] of batch b's
100x100 contrastive matrix, which needs e1 for its 50 own patches
(img branch of feat_c1[b], 5 patch-row slabs) plus e2 for ALL 100
patches (depth branch of feat_c2[b], 10 slabs). Features ship as bf16
(half the DMA bytes; rel err ~4e-3, well under the 2e-2 gate).

The conv1x1 runs transposed-stationary on the PE: the slab pixels are
the stationary lhsT ([128c, 128pix] slice) and the conv weight column
is the moving rhs ([128c, 1]), accumulating over the two channel
halves, so each matmul lands a full xT column [128,1] directly in
PSUM. This removes the [1, N]-layout conv output entirely: no
1-partition evacuation ops, no compaction DMA, no PE transpose.
ReLU+bias happen on the 128-partition xT tile during PSUM->SBUF evac.

Patches are processed in groups in slab-arrival order (own 20+30 |
full 3x early slabs | f3-f5 | f6,f8,f9,f7 last with f7 split across
all 3 DMA queues); the full-branch eT columns are permuted accordingly
and the host unpermutes the logits columns. LayerNorm's rstd is
computed entirely on DVE (reciprocal + min-of-two-chords linear seed +
2 Newton iterations) so no LN step ever queues behind the scalar
engine's DMA backlog; the logits ship in two column strips. No
collectives / cross-core traffic at all (an AllGather costs a flat
15us in this cost regime). sqrt(exp(logit_scale)) is folded into both
branches' LayerNorm affine on the host.
"""
import os
import numpy as np
import ml_dtypes
import concourse.bass as bass
import concourse.bacc as bacc
import concourse.mybir as mybir
import concourse.tile as tile
from concourse.bass_utils import run_bass_kernel_spmd


F32 = mybir.dt.float32
BF16 = mybir.dt.bfloat16
AF = mybir.ActivationFunctionType
ALU = mybir.AluOpType

NV = NH = 10          # patch grid
NP = NV * NH          # 100 patches per batch-modality
CPS = 16
ENC = 128
LN_EPS = 1e-5
HALF = NP // 2        # 50 logits rows per core
NTOT = HALF + NP      # 150 patches per core (50 own + 100 full)

# wpack (bf16): 0:512 own w1t | 512:768 own w2t | 768:1280 full w1t |
#   1280:1536 full w2t.  wsmall (f32): own conv_b, g*s, b*s, then full.

# per-slab chunks (LayerNorm is per-patch, so each 10-patch slab is an
# independent pipeline unit). eT cols: own o_i -> 10i; full f0-f6 ->
# 50+10i, f8 -> 120, f9 -> 130, f7 -> 140 (f7 lands last; host
# unpermutes the logits columns via FULL_PATCH_ORDER).
SLAB_COL = {**{f"o{i}": 10 * i for i in range(5)},
            **{f"f{i}": 50 + 10 * i for i in range(7)},
            "f8": 120, "f9": 130, "f7": 140}
SLAB_N = 10
FULL_PATCH_ORDER = (list(range(0, 70)) + list(range(80, 100))
                    + list(range(70, 80)))
# DVE has no pow/rsqrt in the real ISA. Default: DVE-only reciprocal +
# linear seed + 3 Newton iterations (converges for v in [0.005, 0.5];
# the observed pre-LN variance range is [0.024, 0.139]). BASS_RSTD=sqrt
# selects the scalar-engine Sqrt path instead.
RSTD_SQRT = os.environ.get("BASS_RSTD") == "sqrt"
# rsqrt seed: min of two chords of sqrt(u), u = 1/v, fit on u in [5,15]
# and [15,55]; min-of-chords underestimates (concave), so 2 Newton
# iterations converge to <1e-3 over the whole observed variance range.
RSQRT_CHORDS = [(1.359, 0.1754), (2.093, 0.1142), (3.144, 0.0767)]


def build_kernel(nc, n_cores=8):
    # feats are host-permuted to patch-major, pixel-contiguous layout
    # [u, p, patch, 256] so the conv's stationary AP is a single free dim
    fown = nc.dram_tensor("fown", [2, 128, HALF, 2 * ENC], BF16,
                          kind="ExternalInput")
    ffull = nc.dram_tensor("ffull", [2, 128, NP, 2 * ENC], BF16,
                           kind="ExternalInput")
    convw = nc.dram_tensor("convw", [128, 4], BF16, kind="ExternalInput")
    wpack = nc.dram_tensor("wpack", [128, 1536], BF16, kind="ExternalInput")
    wsmall = nc.dram_tensor("wsmall", [128, 6], F32, kind="ExternalInput")
    logits = nc.dram_tensor("logits", [HALF, NP], F32, kind="ExternalOutput")

    with tile.TileContext(nc) as tc:
        with (
            tc.tile_pool(name="slab", bufs=15) as slab_pool,
            tc.tile_pool(name="cst", bufs=1) as cst,
            tc.tile_pool(name="work", bufs=1) as work,
            tc.tile_pool(name="rot", bufs=7) as rot,
            tc.tile_pool(name="cv", bufs=3, space="PSUM") as ps_cv,
            tc.tile_pool(name="mm", bufs=4, space="PSUM") as ps_mm,
            tc.tile_pool(name="lg", bufs=1, space="PSUM") as ps_lg,
        ):
            convw_s = cst.tile([128, 4], BF16, tag="convw")
            wp_s = cst.tile([128, 1536], BF16, tag="wpack")
            ws_s = cst.tile([128, 6], F32, tag="wsmall")
            ones_col = cst.tile([128, 1], F32, tag="onec")
            ones_row = cst.tile([1, 128], F32, tag="oner")
            nones_row = cst.tile([1, 128], F32, tag="noner")
            scr = cst.tile([1, 8], F32, tag="scr")
            nc.gpsimd.memset(ones_col[:], 1.0 / ENC)  # stats -> means
            nc.gpsimd.memset(ones_row[:], 1.0)
            nc.gpsimd.memset(nones_row[:], -1.0)
            nc.scalar.dma_start(convw_s[:], convw[:])
            if RSTD_SQRT:  # preload the Sqrt activation table early
                nc.scalar.activation(scr[0:1, 0:1], ones_row[0:1, 0:1],
                                     AF.Sqrt)
            nc.scalar.dma_start(wp_s[:], wpack[:])
            nc.scalar.dma_start(ws_s[:], wsmall[:])
            w1t = [wp_s[:, 0:512], wp_s[:, 768:1280]]        # per branch
            w2t = [wp_s[:, 512:768], wp_s[:, 1280:1536]]
            cb_s = [ws_s[:, 0:1], ws_s[:, 3:4]]              # conv bias
            g_s = [ws_s[:, 1:2], ws_s[:, 4:5]]
            b_s = [ws_s[:, 2:3], ws_s[:, 5:6]]

            # persistent sbuf tiles
            xT = [work.tile([128, NTOT], BF16, tag=f"xT{v}", name=f"xT{v}")
                  for v in range(2)]
            hT = [work.tile([128, NTOT], BF16, tag=f"hT{t}", name=f"hT{t}")
                  for t in range(2)]
            yT = work.tile([128, NTOT], F32, tag="yT")
            sqT = work.tile([128, NTOT], F32, tag="sqT")
            eT = work.tile([128, NTOT], F32, tag="eT")
            ego = work.tile([128, HALF], F32, tag="ego")   # g_full * eT_own
            wrow = work.tile([1, 128], F32, tag="wrow")    # [-g^T eTo | b^T eTo]

            slabs = {}
            for nm in [f"o{i}" for i in range(5)] + [f"f{i}" for i in range(10)]:
                slabs[nm] = slab_pool.tile([128, 2, SLAB_N, 2 * ENC], BF16,
                                           tag="slab", name=f"st_{nm}")

            def slab_dma(eng, nm, n0=0, n1=SLAB_N):
                src = fown if nm[0] == "o" else ffull
                s = int(nm[1:])
                eng.dma_start(
                    slabs[nm][:, :, n0:n1, :],
                    src[:, :, s * SLAB_N + n0:s * SLAB_N + n1, :].rearrange(
                        "u p n x -> p u n x"))

            # queue schedule (slab 3948ns, f7 split across all queues):
            #  SP:   o0 o2 f1 f4 f8 f7[0:4]             ~20.9us busy
            #  Pool: o1 o3 f2 f5 f9 f7[4:8]              ~20.9us
            #  Act:  cw wp wsm o4 f0 f3 f6 f7[8:16]      ~21.2us
            for nm in ["o0", "o2", "f1", "f4", "f8"]:
                slab_dma(nc.sync, nm)
            slab_dma(nc.sync, "f7", 0, 3)
            for nm in ["o1", "o3", "f2", "f5", "f9"]:
                slab_dma(nc.gpsimd, nm)
            slab_dma(nc.gpsimd, "f7", 3, 6)
            for nm in ["o4", "f0", "f3", "f6"]:
                slab_dma(nc.scalar, nm)
            slab_dma(nc.scalar, "f7", 6, 10)

            # processing groups: slabs with contiguous eT columns, convolved
            # into one shared psum pair per group, then batched
            # evac/MLP/stats/LN over the whole column range.
            def run_group(subs, ve, cvp=None, raw=False, prows=False):
                cvp = cvp or ps_cv
                nms = [nm for sub in subs for nm in sub]
                n = SLAB_N * len(nms)
                c0 = min(SLAB_COL[nm] for nm in nms)
                br = 0 if nms[0][0] == "o" else 1
                J = slice(c0, c0 + n)
                rb = ps_mm.tile([128, 512], F32, tag="mm", name=f"rb_{nms[0]}")
                off = 0
                for sub in subs:
                    ns = SLAB_N * len(sub)
                    cs = c0 + off
                    Js = slice(cs, cs + ns)
                    pxg = [cvp.tile([128, 512], F32, tag="cv",
                                    name=f"px_{sub[0]}{v}") for v in range(2)]
                    for i, nm in enumerate(sub):
                        st = slabs[nm]
                        for j in range(SLAB_N):
                            for v in range(2):
                                for u in range(2):
                                    # one accumulation group per column: safe
                                    # under any scheduler order (start only
                                    # lazily zeroes; reads see raw psum)
                                    nc.tensor.matmul(
                                        pxg[v][:, 10 * i + j:10 * i + j + 1],
                                        st[:, u, j, 128 * v:128 * (v + 1)],
                                        convw_s[:, 2 * br + u:2 * br + u + 1],
                                        start=(u == 0), stop=(u == 1),
                                    )
                    for v in range(2):
                        ve.tensor_scalar(xT[v][:, Js], pxg[v][:, 0:ns],
                                         cb_s[br][:], 0.0, ALU.add, ALU.max)
                    for t in range(2):
                        ph = ps_mm.tile([128, 512], F32, tag="mm",
                                        name=f"ph{t}_{sub[0]}")
                        for v in range(2):
                            nc.tensor.matmul(
                                ph[:, 0:ns],
                                w1t[br][:, 256 * v + 128 * t:
                                          256 * v + 128 * t + 128],
                                xT[v][:, Js], start=(v == 0), stop=(v == 1))
                        ve.tensor_scalar_max(hT[t][:, Js], ph[:, 0:ns], 0.0)
                    py = ps_mm.tile([128, 512], F32, tag="mm",
                                    name=f"py_{sub[0]}")
                    for t in range(2):
                        nc.tensor.matmul(py[:, 0:ns],
                                         w2t[br][:, 128 * t:128 * t + 128],
                                         hT[t][:, Js], start=(t == 0),
                                         stop=(t == 1))
                    ve.tensor_copy(yT[:, Js], py[:, 0:ns])
                    (nc.gpsimd if raw else ve).tensor_tensor(
                        sqT[:, Js], yT[:, Js], yT[:, Js], ALU.mult)
                    nc.tensor.matmul(rb[0:1, off:off + ns], ones_col[:],
                                     yT[:, Js], start=True, stop=True)
                    nc.tensor.matmul(rb[0:1, n + off:n + off + ns],
                                     ones_col[:], sqT[:, Js],
                                     start=True, stop=True)
                    off += ns
                rw = rot.tile([1, 320], F32, tag="rw", name=f"rw_{nms[0]}")
                qrow = rb[0:1, n:2 * n]
                mrow = rw[0:1, 0:n]
                rstd = rw[0:1, 120:120 + n]
                t1, veps = rw[0:1, 184:184 + n], rw[0:1, 248:248 + n]
                ve.tensor_copy(mrow, rb[0:1, 0:n])   # mean to SBUF
                ve.tensor_tensor(t1, mrow, mrow, ALU.mult)
                # eps (1e-5) is <0.05% of the observed minimum variance
                # (0.024); folding it away costs ~2e-4 rel on rstd
                ve.tensor_tensor(veps, qrow, t1, ALU.subtract)
                nc.vector.reciprocal(t1, veps)
                if raw or prows:
                    # late group: single-chord seed + 2 Newton iterations,
                    # all on the idle gpsimd queue (TensorTensor min is not
                    # legal on Pool, so no min-of-chords here)
                    re, iters = nc.gpsimd, 2
                    re.tensor_scalar(rstd, t1, 0.1036, 1.718,
                                     ALU.mult, ALU.add)
                else:
                    re, iters = ve, 1
                    s2 = rw[0:1, 60:60 + n]
                    a0, b0 = RSQRT_CHORDS[0]
                    ve.tensor_scalar(rstd, t1, b0, a0, ALU.mult, ALU.add)
                    for a, b in RSQRT_CHORDS[1:]:
                        ve.tensor_scalar(s2, t1, b, a, ALU.mult, ALU.add)
                        ve.tensor_tensor(rstd, rstd, s2, ALU.min)
                for _ in range(iters):
                    re.tensor_tensor(t1, rstd, rstd, ALU.mult)
                    re.tensor_tensor(t1, t1, veps, ALU.mult)
                    re.tensor_scalar(t1, t1, -0.5, 1.5, ALU.mult, ALU.add)
                    re.tensor_tensor(rstd, rstd, t1, ALU.mult)
                re.tensor_tensor(mrow, mrow, rstd, ALU.mult)  # mean*rstd
                nc.tensor.matmul(rb[:, 128:128 + n], ones_row[:], rstd,
                                 start=True, stop=True)
                if raw:
                    # eT holds y*rstd only; -mean*rstd and +b are folded
                    # into the logits matmul as rank-1 accumulations
                    ve.tensor_tensor(eT[:, J], yT[:, J],
                                     rb[:, 128:128 + n], ALU.mult)
                    return rw
                nc.tensor.matmul(rb[:, 192:192 + n], nones_row[:],
                                 mrow, start=True, stop=True)
                ve.tensor_tensor(yT[:, J], yT[:, J],
                                 rb[:, 128:128 + n], ALU.mult)
                ve.tensor_tensor(yT[:, J], yT[:, J],
                                 rb[:, 192:192 + n], ALU.add)
                ve.tensor_scalar(eT[:, J], yT[:, J], g_s[br][:],
                                 b_s[br][:], ALU.mult, ALU.add)
                return rw

            # early groups ride DVE; late groups ride gpsimd, whose DMA
            # queue drains just before their slabs land
            run_group([["o0", "o1"]], nc.vector)
            run_group([["o2", "o3", "o4"]], nc.vector)
            # folded-logits precomputes (own eT ready; full-branch g/b)
            nc.vector.tensor_scalar(ego[:, 0:HALF], eT[:, 0:HALF],
                                    g_s[1][:], None, ALU.mult)
            pwr = ps_mm.tile([128, 512], F32, tag="mm", name="pwr")
            nc.tensor.matmul(pwr[0:1, 0:HALF], g_s[1], eT[:, 0:HALF],
                             start=True, stop=True)
            nc.tensor.matmul(pwr[0:1, 64:64 + HALF], b_s[1], eT[:, 0:HALF],
                             start=True, stop=True)
            nc.vector.tensor_scalar(wrow[0:1, 0:HALF], pwr[0:1, 0:HALF],
                                    -1.0, None, ALU.mult)
            nc.vector.tensor_copy(wrow[0:1, 64:64 + HALF],
                                  pwr[0:1, 64:64 + HALF])
            run_group([["f0", "f1", "f2"]], nc.vector)
            run_group([["f3", "f4", "f5"]], nc.vector, prows=True)

            with tc.high_priority():
                rwL = run_group([["f6", "f8", "f9", "f7"]], nc.vector,
                                raw=True)

            # logits part 1: device cols 0:60 (f0-f5)
            pL = ps_lg.tile([HALF, NP], F32, tag="lg")
            L_s = work.tile([HALF, NP], F32, tag="Ls")
            nc.tensor.matmul(pL[:, 0:60], eT[:, 0:HALF], eT[:, HALF:HALF + 60],
                             start=True, stop=True)
            nc.scalar.copy(L_s[:, 0:60], pL[:, 0:60])
            nc.scalar.dma_start(logits[:, 0:60], L_s[:, 0:60])

            nc.tensor.matmul(pL[:, 60:NP], wrow[0:1, 64:64 + HALF],
                             ones_row[0:1, 0:40], start=True, stop=False)
            nc.tensor.matmul(pL[:, 60:NP], wrow[0:1, 0:HALF],
                             rwL[0:1, 0:40], start=False, stop=False)
            nc.tensor.matmul(pL[:, 60:NP], ego[:, 0:HALF],
                             eT[:, HALF + 60:NTOT], start=False, stop=True)
            nc.vector.tensor_copy(L_s[:, 60:NP], pL[:, 60:NP])
            nc.sync.dma_start(logits[:, 60:NP], L_s[:, 60:NP])

    nc.compile()
    return nc


def host_inputs_for_core(core, inputs):
    """Build the per-core in_map from the full problem inputs dict."""
    b, half = core // 2, core % 2
    f1 = np.asarray(inputs["feat_c1"])[b]          # [256,160,160] img branch
    f2 = np.asarray(inputs["feat_c2"])[b]          # depth branch
    fown = f1[:, half * 80:half * 80 + 80, :]      # own 5 patch-rows
    s_half = np.sqrt(np.exp(np.float32(np.asarray(inputs["logit_scale"]))))

    def conv2(pre):
        return np.asarray(inputs[pre + "conv_w"]).reshape(2, 128).T  # [128,2]

    convw = np.concatenate([conv2("img_"), conv2("depth_")], axis=1)

    def w1t(pre):
        w1 = np.asarray(inputs[pre + "w1"])        # [256,256]
        return np.ascontiguousarray(
            w1.reshape(2, 128, 2, 128).transpose(3, 2, 0, 1).reshape(128, 512))

    def w2t(pre):
        w2 = np.asarray(inputs[pre + "w2"])        # [128,256]
        return np.ascontiguousarray(
            w2.reshape(128, 2, 128).transpose(2, 1, 0).reshape(128, 256))

    def col(val):
        return np.full((128, 1), np.float32(val), np.float32)

    def gcol(pre, nm):
        return (np.asarray(inputs[pre + nm]) * s_half).reshape(128, 1)

    wpack = np.concatenate([
        w1t("img_"), w2t("img_"), w1t("depth_"), w2t("depth_"),
    ], axis=1)
    wsmall = np.concatenate([
        col(np.asarray(inputs["img_conv_b"])[0]),
        gcol("img_", "ln_g").astype(np.float32),
        gcol("img_", "ln_b").astype(np.float32),
        col(np.asarray(inputs["depth_conv_b"])[0]),
        gcol("depth_", "ln_g").astype(np.float32),
        gcol("depth_", "ln_b").astype(np.float32),
    ], axis=1).astype(np.float32)
    def patchmaj(f, nr):
        # [256, nr*16, 160] -> [2, 128, nr*10 patches, 256 pixels]
        t = f.reshape(2, 128, nr, CPS, NH, CPS).transpose(0, 1, 2, 4, 3, 5)
        return np.ascontiguousarray(t.reshape(2, 128, nr * NH, 2 * ENC))

    return {
        "fown": patchmaj(fown, 5).astype(ml_dtypes.bfloat16),
        "ffull": patchmaj(f2, 10).astype(ml_dtypes.bfloat16),
        "convw": convw.astype(ml_dtypes.bfloat16),
        "wpack": np.ascontiguousarray(wpack).astype(ml_dtypes.bfloat16),
        "wsmall": np.ascontiguousarray(wsmall),
    }


_NC_CACHE = {}


def _get_nc():
    if "nc" not in _NC_CACHE:
        nc = bacc.Bacc("TRN2", target_bir_lowering=False, num_devices=8)
        build_kernel(nc)
        _NC_CACHE["nc"] = nc
    return _NC_CACHE["nc"]


def kernel(**inputs):
    nc = _get_nc()
    in_maps = [host_inputs_for_core(c, inputs) for c in range(8)]
    res = run_bass_kernel_spmd(nc, in_maps, list(range(8)))
    perm = np.asarray(FULL_PATCH_ORDER)
    logits_img = np.empty((4, NP, NP), np.float32)
    for b in range(4):
        dev = np.concatenate([np.asarray(res.results[2 * b]["logits"]),
                              np.asarray(res.results[2 * b + 1]["logits"])],
                             axis=0).astype(np.float32)
        logits_img[b][:, perm] = dev
    logits_depth = np.ascontiguousarray(logits_img.transpose(0, 2, 1))
    return logits_img, logits_depth
